# revision 35
# baseline (speedup 1.0000x reference)
"""InvGridSamplerNumerator kernel for 8x TRN2 NeuronCores.

Batch-parallel over 8 cores (B=8). The bilinear splat (scatter-add with
random collisions) is restructured as a dense segmented reduction:

  host:   one pixel-level stable sort by base output cell; the 4 taps per
          pixel (2x2 bilinear stencil) form 4 streams that share that pixel
          order, so per-cell slot positions for all 1M taps come from
          vectorized shifted-count arithmetic (no tap-level sort). Cells are
          padded to rows of 4 slots; taps cropped by the output window get
          zero weight or land in an extended cell range that is dropped at
          placement. Streams are packed in bfloat16 to halve wire volume.
  device: per slot, multiply the 16-channel x vector by the tap weight
          (DVE), then sum each row's 4 slots -> one 16-channel partial per
          row. Triple-buffered streaming; plain DMA only.
  host:   place row partials into the (C,H,W) image with one per-channel
          bincount (cells with >4 taps have multiple rows; bincount
          accumulates them).
"""
import numpy as np
import ml_dtypes

B, C, H, W = 8, 16, 512, 512
NBC = H * W            # base cells
ECELL = NBC + 513      # extended cell space (row-cropped taps -> >= NBC)
S = 4                  # slots per row
R = 128                # rows per partition per tile
TILE_ROWS = 128 * R
EPS = 1e-10
BF16 = ml_dtypes.bfloat16

_cache = {}


def _build(nt: int):
    import concourse.bass as bass
    import concourse.bacc as bacc
    import concourse.mybir as mybir

    nc = bacc.Bacc(None, target_bir_lowering=False)
    xs_in = nc.dram_tensor("xs", [nt, 128, R * S * C], mybir.dt.int8, kind="ExternalInput")
    w_in = nc.dram_tensor("w", [nt, 128, R * S], mybir.dt.bfloat16, kind="ExternalInput")
    rs_in = nc.dram_tensor("rs", [nt, 128, R], mybir.dt.bfloat16, kind="ExternalInput")
    rows_out = nc.dram_tensor("rows", [nt, 128, R * C], mybir.dt.int8, kind="ExternalOutput")

    NB = 3  # buffer slots
    with (
        nc.Block() as block,
        nc.semaphore("ld") as ld,
        nc.semaphore("pv") as pv,
        nc.semaphore("so") as so,
        nc.sbuf_tensor("xt", [128, NB * R * S * C], mybir.dt.int8) as xt,
        nc.sbuf_tensor("wt", [128, NB * R * S], mybir.dt.bfloat16) as wt,
        nc.sbuf_tensor("tm", [128, NB * R * S * C], mybir.dt.bfloat16) as tm,
        nc.sbuf_tensor("ot", [128, NB * R * C], mybir.dt.bfloat16) as ot,
        nc.sbuf_tensor("qt", [128, NB * R * C], mybir.dt.int8) as qt,
        nc.sbuf_tensor("mx", [128, NB * R], mybir.dt.bfloat16) as mx,
    ):
        def xv(b):  # [128, R*S, C] view of buffer b
            return xt[:, b * R * S * C:(b + 1) * R * S * C].rearrange("p (n c) -> p n c", c=C)

        def wv(b):  # [128, R*S] view
            return wt[:, b * R * S:(b + 1) * R * S]

        def tv(b):
            return tm[:, b * R * S * C:(b + 1) * R * S * C].rearrange("p (n c) -> p n c", c=C)

        def ov(b):
            return ot[:, b * R * C:(b + 1) * R * C]

        def qv(b):
            return qt[:, b * R * C:(b + 1) * R * C]

        def mv(b):
            return mx[:, b * R:(b + 1) * R]

        @block.sync
        def _(sync):
            for t in range(nt):
                b = t % NB
                if t >= NB:
                    sync.wait_ge(pv, t - NB + 1)
                sync.dma_start(xv(b).rearrange("p n c -> p (n c)"), xs_in[t]).then_inc(ld, 16)
                sync.dma_start(wv(b), w_in[t]).then_inc(ld, 16)
                sync.dma_start(mv(b), rs_in[t]).then_inc(ld, 16)

        @block.vector
        def _(vector):
            for t in range(nt):
                b = t % NB
                vector.wait_ge(ld, 48 * (t + 1))
                if t >= NB:
                    vector.wait_ge(so, 16 * (t - NB + 1))
                # tmp = x * w (w broadcast along channel dim)
                vector.tensor_mul(tv(b), xv(b), wv(b)[:, :, None].to_broadcast([128, R * S, C]))
                # reduce 4 slots per row: view tmp as [128, R, S, C]
                t4 = tv(b).rearrange("p (r s) c -> p r s c", s=S)
                o3 = ov(b).rearrange("p (r c) -> p r c", c=C)
                vector.tensor_add(t4[:, :, 0, :], t4[:, :, 0, :], t4[:, :, 1, :])
                vector.tensor_add(t4[:, :, 2, :], t4[:, :, 2, :], t4[:, :, 3, :])
                vector.tensor_add(o3, t4[:, :, 0, :], t4[:, :, 2, :])
                # int8-quantize rows with host-computed reciprocal row scale
                m2 = mv(b).rearrange("p (r u) -> p r u", u=1)
                d3 = tv(b)[:, :R, :]  # reuse product scratch as bf16 staging
                vector.tensor_mul(d3, o3, m2.to_broadcast([128, R, C]))
                vector.tensor_copy(qv(b), d3.rearrange("p r c -> p (r c)")).then_inc(pv, 1)

        @block.gpsimd
        def _(gpsimd):
            for t in range(nt):
                b = t % NB
                gpsimd.wait_ge(pv, t + 1)
                gpsimd.dma_start(rows_out[t], qv(b)).then_inc(so, 16)
            gpsimd.wait_ge(so, 16 * nt)

    nc.finalize()
    return nc


def _host_prep(inv_grid_b):
    """Pixel sort + vectorized slot assignment for all 4 tap streams."""
    g = (inv_grid_b.astype(np.float32) + np.float32(1.0)) * np.float32(0.5)
    gi = np.clip(g[..., 0] * np.float32(H) + np.float32(1.0), np.float32(0.0),
                 np.float32(H + 1 - 2 * EPS)).reshape(-1)
    gj = np.clip(g[..., 1] * np.float32(W) + np.float32(1.0), np.float32(0.0),
                 np.float32(W + 1 - 2 * EPS)).reshape(-1)
    fi = np.floor(gi).astype(np.int32)
    fj = np.floor(gj).astype(np.int32)
    wi1 = gi - fi
    wi0 = np.float32(1.0) - wi1
    wj1 = (gj - fj) * (fj != W)  # col-cropped dj=1 taps wrap: zero them
    wj0 = np.float32(1.0) - (gj - fj)
    bcell = (fi - 1) * np.int32(W) + (fj - 1)

    order = np.argsort(bcell)
    bs = bcell[order]
    cnt = np.bincount(bcell, minlength=NBC).astype(np.int64)
    start = np.zeros(NBC + 1, np.int64)
    np.cumsum(cnt, out=start[1:])
    rank = np.arange(NBC, dtype=np.int64) - start[bs]

    cntE = np.zeros(ECELL, np.int64)
    cntE[:NBC] = cnt
    tot = cntE.copy()
    offs = (0, 1, W, W + 1)
    qoff = [None, None, None, None]
    for q, off in enumerate(offs[1:], start=1):
        qoff[q] = tot.copy()
        tot[off:] += cntE[:ECELL - off]
    nr = (tot + S - 1) // S
    row_start = np.zeros(ECELL + 1, np.int64)
    np.cumsum(nr, out=row_start[1:])
    NR = int(row_start[-1])

    wq_all = (wi0 * wj0, wi0 * wj1, wi1 * wj0, wi1 * wj1)
    slot_of = np.empty((4, NBC), np.int64)
    for q, off in enumerate(offs):
        c = bs + off
        base = row_start[c] * S + rank
        if q:
            base += qoff[q][c]
        slot_of[q] = base
    return order, slot_of, wq_all, nr, NR


def _build_streams(x_b, prep, nt):
    order, slot_of, wq_all, nr, NR = prep
    nslot_pad = nt * TILE_ROWS * S
    # int8-quantize each pixel vector; fold the per-pixel scale into w.
    # Quantize channel-major first so the pixel-major transpose moves 4x
    # fewer bytes (int8 vs f32).
    x2d = x_b.reshape(C, NBC)
    amax = np.abs(x2d).max(axis=0)
    scale = amax * np.float32(1.0 / 127.0)
    inv = np.float32(127.0) / np.maximum(amax, np.float32(1e-30))
    q8_ch = np.clip(np.rint(x2d * inv[None, :]), -127, 127).astype(np.int8)
    q8o = np.ascontiguousarray(q8_ch.T)[order]     # pixel vectors, sorted
    scale_o = scale[order]
    xs = np.zeros((nslot_pad, C), np.int8)
    wf = np.zeros(nslot_pad, np.float32)
    for q in range(4):
        xs[slot_of[q]] = q8o
        wf[slot_of[q]] = wq_all[q][order] * scale_o
    wv = wf.astype(BF16).reshape(nt, 128, R * S)
    # per-row output bound: |row| <= 127 * sum(w); device maps rows into
    # int8 range via rs = (127*0.98)/bound (0.98 absorbs bf16 rounding so
    # the int8 cast cannot clip); host multiplies back by bound/(127*0.98)
    bound = np.float32(127.0) * wf.reshape(-1, S).sum(axis=1)
    K = np.float32(127.0 * 0.98)
    rs = (K / np.maximum(bound, np.float32(1e-20))).astype(BF16)
    return xs.reshape(nt, 128, R * S * C), wv, rs.reshape(nt, 128, R), bound * np.float32(1.0) / K


def _place(rows_f32, nr, NR):
    row_cell = np.repeat(np.arange(ECELL, dtype=np.int64), nr)
    keep = row_cell < NBC
    rc = row_cell[keep]
    rv = rows_f32[:NR][keep]
    out = np.empty((C, NBC), np.float32)
    for c in range(C):
        out[c] = np.bincount(rc, weights=rv[:, c], minlength=NBC)[:NBC]
    return out.reshape(C, H, W)


def _run_spmd_fast(nc, feed, nt):
    """Sharded PJRT run with per-batch async device_put (overlaps transfers
    with host stream building via `feed`) and on-device zero-initialized
    output buffers — avoids run_bass_kernel_spmd's host-side concat and
    shipping zeros over the wire.

    `feed(b)` returns the in_map dict for core b; transfers start as soon as
    each batch's arrays are built.
    """
    import jax
    import jax.numpy as jnp
    import concourse.mybir as mybir
    from concourse import bass2jax
    from jax.sharding import Mesh, NamedSharding, PartitionSpec
    from jax.experimental.shard_map import shard_map

    bass2jax.install_neuronx_cc_hook()
    assert nc.dbg_addr is None
    partition_name = (
        nc.partition_id_tensor.name if nc.partition_id_tensor else None
    )

    in_names, out_names, out_avals = [], [], []
    for alloc in nc.m.functions[0].allocations:
        if not isinstance(alloc, mybir.MemoryLocationSet):
            continue
        name = alloc.memorylocations[0].name
        if alloc.kind == "ExternalInput":
            if name != partition_name:
                in_names.append(name)
        elif alloc.kind == "ExternalOutput":
            out_avals.append(
                jax.core.ShapedArray(tuple(alloc.tensor_shape), mybir.dt.np(alloc.dtype))
            )
            out_names.append(name)
    n_params = len(in_names)
    all_in_names = list(in_names) + list(out_names)
    if partition_name is not None:
        all_in_names.append(partition_name)

    devices = jax.devices()[:B]
    mesh = Mesh(np.asarray(devices), ("core",))
    sh = NamedSharding(mesh, PartitionSpec("core"))

    def _body(*args):
        operands = list(args)
        if partition_name is not None:
            operands.append(bass2jax.partition_id_tensor())
        return tuple(
            bass2jax._bass_exec_p.bind(
                *operands,
                out_avals=tuple(out_avals),
                in_names=tuple(all_in_names),
                out_names=tuple(out_names),
                lowering_input_output_aliases=(),
                sim_require_finite=True,
                sim_require_nnan=True,
                nc=nc,
            )
        )

    donate = tuple(range(n_params, n_params + len(out_names)))
    sharded = jax.jit(
        shard_map(_body, mesh=mesh,
                  in_specs=(PartitionSpec("core"),) * (n_params + len(out_names)),
                  out_specs=(PartitionSpec("core"),) * len(out_names),
                  check_rep=False),
        donate_argnums=donate, keep_unused=True,
    )

    # per-core async transfers, started as each batch's streams are built
    shards = [[None] * B for _ in in_names]
    for b in range(B):
        m = feed(b)
        for i, name in enumerate(in_names):
            shards[i][b] = jax.device_put(m[name], devices[b])
    globals_in = []
    for i in range(n_params):
        pshape = tuple(shards[i][0].shape)
        globals_in.append(
            jax.make_array_from_single_device_arrays(
                (B * pshape[0],) + pshape[1:], sh, shards[i]
            )
        )
    zeros = [
        jax.jit(lambda shape=tuple(av.shape), dt=av.dtype:
                jnp.zeros((B * shape[0],) + shape[1:], dt),
                out_shardings=sh)()
        for av in out_avals
    ]
    out_arrs = sharded(*globals_in, *zeros)
    return [
        {name: np.asarray(out_arrs[i]).reshape(B, *out_avals[i].shape)[c]
         for i, name in enumerate(out_names)}
        for c in range(B)
    ]


def kernel(x: np.ndarray, inv_grid: np.ndarray) -> np.ndarray:
    x = np.asarray(x, dtype=np.float32)
    inv_grid = np.asarray(inv_grid, dtype=np.float32)

    preps = [_host_prep(inv_grid[b]) for b in range(B)]
    nt = (max(p[4] for p in preps) + TILE_ROWS - 1) // TILE_ROWS

    if nt not in _cache:
        _cache[nt] = _build(nt)
    nc = _cache[nt]

    dequants = [None] * B

    def feed(b):
        xs, wv, rs, dq = _build_streams(x[b], preps[b], nt)
        dequants[b] = dq
        return {"xs": xs, "w": wv, "rs": rs}

    try:
        results = _run_spmd_fast(nc, feed, nt)
    except Exception:
        from concourse.bass_utils import run_bass_kernel_spmd
        in_maps = [feed(b) for b in range(B)]
        results = run_bass_kernel_spmd(nc, in_maps, core_ids=list(range(B))).results

    out = np.empty((B, C, H, W), np.float32)
    for b in range(B):
        _, _, _, nr, NR = preps[b]
        q = np.asarray(results[b]["rows"]).reshape(-1, C)
        rows = np.multiply(q, dequants[b][:, None], dtype=np.float32)
        out[b] = _place(rows, nr, NR)
    return out
